# revision 25
# baseline (speedup 1.0000x reference)
"""NeuralODE Euler-integration kernel for 8 TRN2 NeuronCores (v2).

Problem: y' = MLP(y), MLP = Linear(64,256)+ReLU, Linear(256,256)+ReLU,
Linear(256,64); Euler y_{t+1} = y_t + dt*MLP(y_t), T=200 (199 steps),
B=4096, output [B, 200, 64] with slot 0 = y0.

Sharding: pure data-parallel, 512 batch rows per core, weights replicated.

v2 design — keep the serial recurrence entirely on the PE:
  * a1 := W1^T y (pre-activation of layer 1) lives in PSUM across all
    steps. Identity: a1_{t+1} = a1_t + (dt*W3@W1)^T h2_t + dt*W1^T b3,
    realized as matmuls with start=False (PSUM has_written bits persist),
    so the critical loop is  MM(a1+=) -> relu(h1) -> MM(ps2) -> relu(h2)
    -> MM(a1+=)...  The Euler y-update never enters the chain.
  * ps_y := y accumulates dt*W3^T h2 increments in another persistent
    PSUM bank (also start=False matmuls). One copy op per step moves
    ps_y -> SBUF (fp32, exact) for the DMA of y_t. No bf16 state shadow.
  * Batch split into S=2 independent streams of 256 cols; layers of the
    two streams are interleaved so PE/ACT/DVE always have work.
  * h1 is produced chunk-split (ACT does units 0:128, DVE 128:256, from
    separate PSUM banks -> parallel, low latency); h2 is produced by ONE
    merged op per stream ([128, 2, 256] from a single packed bank),
    alternating ACT/DVE, which minimizes fixed per-op overhead.
  * Init matmuls use float32r (exact fp32 at 1 cycle/col for N>=256):
    a1_0 = W1^T y0, ps_y_0 = I64 @ y0.
  * nwarm0 dummy matmuls before the loop ramp the PE p-state
    (0.65 -> 1.2 -> 2.4 GHz with sustained busy).

PSUM banks: a1 4 (one per stream x chunk), ps2 2 (mc chunks packed per
stream), ps_y 1 (both streams' columns), warm 1.

Measured on HW (traced): ~635 us vs 1041 us baseline (untraced) /
1250 us (traced); rel err 1.09e-3. Step ~3.08 us: PE ~2.3 us busy at
full 2.4 GHz, ACT ~2.0 us, DVE ~2.1 us; remaining gap is the serial
chain h2 -> a1 at the step boundary (~2.64 us chain floor).
Scheduling variants that measured SLOWER (do not retry): splitting h2
across ACT+DVE from one packed bank (PSUM same-bank contention, +145us),
phase-interleaved per-stream emission (+120us), psy matmuls after L2
(+75us), nfill dummy matmuls (no effect; PE clock already ramped),
walrus --enable-ldw-opt=true (codegen crash), matmul tile_position
col-offset 64 to pack psy on 128 partitions (walrus rejects dst
partition base 64).
"""
import numpy as np

import concourse.bass as bass
import concourse.tile as tile
from concourse import bacc, mybir
from concourse.bass_utils import run_bass_kernel_spmd

F32 = mybir.dt.float32
F32R = mybir.dt.float32r
BF16 = mybir.dt.bfloat16
RELU = mybir.ActivationFunctionType.Relu
COPY = mybir.ActivationFunctionType.Copy

B, D, H, T = 4096, 64, 256, 200
NCORES = 8
BL = B // NCORES          # 512 batch rows per core
S = 2                     # independent batch streams per core
FD = BL // S              # 256 cols per stream

_cache = {}


def build(nsteps: int, nwarm0: int = 24, nfill: int = 0, has_b3: bool = False,
          b2_uniform: bool = True):
    add = mybir.AluOpType.add
    mx = mybir.AluOpType.max
    mult = mybir.AluOpType.mult

    nc = bacc.Bacc("TRN2", target_bir_lowering=False, debug=False)
    y0T_d = nc.dram_tensor("y0T", [D, BL], F32R, kind="ExternalInput")
    w1_d = nc.dram_tensor("w1", [D, 2, 128], F32R, kind="ExternalInput")
    i64_d = nc.dram_tensor("i64", [D, D], F32R, kind="ExternalInput")
    w31_d = nc.dram_tensor("w31", [128, 2, 2, 128], BF16, kind="ExternalInput")
    w2_d = nc.dram_tensor("w2", [128, 2, 2, 128], BF16, kind="ExternalInput")
    w3_d = nc.dram_tensor("w3", [128, 2, D], BF16, kind="ExternalInput")
    b1_d = nc.dram_tensor("b1r", [128, 2], F32, kind="ExternalInput")
    b2_d = nc.dram_tensor("b2r", [128, 2], F32, kind="ExternalInput")
    if has_b3:
        # c = dt*W1^T b3 (a1 increment), db3 = dt*b3 (ps_y increment)
        c_d = nc.dram_tensor("crow", [1, 2, 128], F32R, kind="ExternalInput")
        db3_d = nc.dram_tensor("db3row", [1, D], F32R, kind="ExternalInput")
    out_d = nc.dram_tensor("out", [nsteps, D, BL], F32, kind="ExternalOutput")

    with tile.TileContext(nc) as tc:
        with tc.tile_pool(name="wpool", bufs=1) as wp, \
             tc.tile_pool(name="hpool", bufs=16) as hp, \
             tc.tile_pool(name="ypool", bufs=8) as yp, \
             tc.tile_pool(name="ps", bufs=1, space="PSUM") as pp:

            y0T = wp.tile([D, BL], F32R)
            w1 = wp.tile([D, 2, 128], F32R)
            i64 = wp.tile([D, D], F32R)
            w31 = wp.tile([128, 2, 2, 128], BF16)
            w2 = wp.tile([128, 2, 2, 128], BF16)
            w3 = wp.tile([128, 2, D], BF16)
            b1 = wp.tile([128, 2], F32)
            b2 = wp.tile([128, 2], F32)
            nc.sync.dma_start(y0T[:], y0T_d.ap())
            nc.sync.dma_start(w1[:], w1_d.ap())
            nc.sync.dma_start(i64[:], i64_d.ap())
            nc.sync.dma_start(w31[:], w31_d.ap())
            nc.sync.dma_start(w2[:], w2_d.ap())
            nc.sync.dma_start(w3[:], w3_d.ap())
            nc.sync.dma_start(b1[:], b1_d.ap())
            nc.sync.dma_start(b2[:], b2_d.ap())
            if has_b3:
                crow = wp.tile([1, 2, 128], F32R)
                db3row = wp.tile([1, D], F32R)
                ones = wp.tile([1, FD], F32R)
                nc.sync.dma_start(crow[:], c_d.ap())
                nc.sync.dma_start(db3row[:], db3_d.ap())
                nc.gpsimd.memset(ones[:], 1.0)

            # Persistent PSUM state. a1 chunks get a full bank each so the
            # ACT/DVE h1 reads never share a bank.
            a1 = [[pp.tile([128, 2, FD], F32, tag=f"a1_{s}_{m}",
                           name=f"a1_{s}_{m}") for m in range(2)]
                  for s in range(S)]
            # stream 0: one full bank per mc chunk so its h2 can be
            # chunk-split across ACT+DVE without same-bank contention
            # (stream 0's a1 matmuls lead the PE queue, so its h2 gates the
            # step boundary). stream 1 keeps the packed bank + merged h2.
            ps2a = pp.tile([128, 2, FD], F32, tag="ps2a", name="ps2a")
            ps2b = pp.tile([128, 2, FD], F32, tag="ps2b", name="ps2b")
            ps2s1 = pp.tile([128, 2, FD], F32, tag="ps2s1", name="ps2s1")
            psy = pp.tile([D, S, FD], F32, tag="psy", name="psy")

            out_ap = out_d.ap()
            warm_rhs = w2[:, 0, :, :].rearrange("p a b -> p (a b)")

            def fill(n):
                # warm matmuls scribble into psy BEFORE its start=True init,
                # which wipes them -- no dedicated warm bank needed
                for _ in range(n):
                    nc.tensor.matmul(psy[:, 0, :], w2[:, 0, 0, 0:D], warm_rhs,
                                     start=True, stop=True,
                                     skip_group_check=True)

            fill(nwarm0)

            # ps_y init: ONE start=True matmul covering the whole bank
            # (a second start=True into the same bank would clear the first
            # stream's has_written bits and break persistent accumulation)
            nc.tensor.matmul(psy.rearrange("p a b -> p (a b)"), i64[:],
                             y0T[:], start=True, stop=True)

            h2p = [None] * S

            def emit_h1(s):
                h1 = hp.tile([128, 2, FD], BF16, tag="h", name="h1")
                nc.scalar.activation(h1[:, 0, :], a1[s][0][:, 0, :], RELU,
                                     bias=b1[:, 0:1], scale=1.0)
                nc.vector.tensor_scalar(h1[:, 1, :], a1[s][1][:, 0, :],
                                        b1[:, 1:2], 0.0, op0=add, op1=mx)
                return h1

            def emit_l2(s, h1):
                for mc in range(2):
                    dst = (ps2a if mc == 0 else ps2b)[:, 0, :] if s == 0 \
                        else ps2s1[:, mc, :]
                    for kc in range(2):
                        nc.tensor.matmul(dst, w2[:, kc, mc, :], h1[:, kc, :],
                                         start=(kc == 0), stop=(kc == 1))

            def emit_h2(s, t):
                h2 = hp.tile([128, 2, FD], BF16, tag="h", name="h2")
                if s == 0:
                    # chunk-split from SEPARATE banks: ACT and DVE run in
                    # parallel, so h2c0 (which gates a1_{t+1} kc0) lands
                    # ~200ns earlier than a merged op would
                    nc.scalar.activation(h2[:, 0, :], ps2a[:, 0, :], RELU,
                                         bias=b2[:, 0:1], scale=1.0)
                    nc.vector.tensor_scalar(h2[:, 1, :], ps2b[:, 0, :],
                                            b2[:, 1:2], 0.0, op0=add, op1=mx)
                elif b2_uniform:
                    # ONE merged op reading the packed bank: splitting it
                    # across engines would serialize on the shared bank.
                    # Needs b2's halves identical (zeros here).
                    h2f = h2.rearrange("p a b -> p (a b)")
                    p2f = ps2s1.rearrange("p a b -> p (a b)")
                    if t % 2 == 0:
                        nc.scalar.activation(h2f, p2f, RELU,
                                             bias=b2[:, 0:1], scale=1.0)
                    else:
                        nc.vector.tensor_scalar(h2f, p2f, b2[:, 0:1], 0.0,
                                                op0=add, op1=mx)
                else:
                    nc.scalar.activation(h2[:, 0, :], ps2s1[:, 0, :],
                                         RELU, bias=b2[:, 0:1], scale=1.0)
                    nc.vector.tensor_scalar(h2[:, 1, :], ps2s1[:, 1, :],
                                            b2[:, 1:2], 0.0,
                                            op0=add, op1=mx)
                h2p[s] = h2

            # ---- main loop, step-batched emission ----
            # psy group g (produces y_g from h2(g-1)) is emitted at the TOP
            # of iteration g+1: there h2(g-1) is two iterations old, so the
            # group is dependency-free PE filler that absorbs the step-
            # boundary stall (a1 waiting on h2). copy_g/DMA follow it.
            h2hist = {}

            def emit_psy_copy(g):
                for s in range(S):
                    for kc in range(2):
                        nc.tensor.matmul(psy[:, s, :], w3[:, kc, :],
                                         h2hist[g - 1][s][:, kc, :],
                                         start=False, stop=(kc == 1),
                                         skip_group_check=True)
                    if has_b3:
                        nc.tensor.matmul(psy[:, s, :], db3row[:], ones[:],
                                         start=False, stop=True,
                                         skip_group_check=True)
                del h2hist[g - 1]
                yo = yp.tile([D, S, FD], F32, tag="yo", name="yo")
                yof = yo.rearrange("p a b -> p (a b)")
                pyf = psy.rearrange("p a b -> p (a b)")
                if g % 2 == 0:
                    nc.scalar.activation(yof, pyf, COPY)
                else:
                    nc.vector.tensor_scalar(yof, pyf, 1.0, 0.0,
                                            op0=mult, op1=add)
                nc.sync.dma_start(out_ap[g - 1], yof)

            for t in range(nsteps + 1):
                last = t == nsteps
                if t >= 2:
                    emit_psy_copy(t - 1)
                if t == 0:
                    for s in range(S):
                        cs = bass.ts(s, FD)
                        for mc in range(2):
                            nc.tensor.matmul(a1[s][mc][:, 0, :], w1[:, mc, :],
                                             y0T[:, cs], start=True, stop=True)
                elif not last:
                    for s in range(S):
                        for mc in range(2):
                            for kc in range(2):
                                nc.tensor.matmul(a1[s][mc][:, 0, :],
                                                 w31[:, kc, mc, :],
                                                 h2p[s][:, kc, :],
                                                 start=False, stop=(kc == 1),
                                                 skip_group_check=True)
                        if has_b3:
                            nc.tensor.matmul(a1[s][0][:, 0, :], crow[:, 0, :],
                                             ones[:], start=False, stop=True,
                                             skip_group_check=True)
                            nc.tensor.matmul(a1[s][1][:, 0, :], crow[:, 1, :],
                                             ones[:], start=False, stop=True,
                                             skip_group_check=True)
                if not last:
                    h1s = [emit_h1(s) for s in range(S)]
                    for s in range(S):
                        emit_l2(s, h1s[s])
                    for s in range(S):
                        emit_h2(s, t)
                    h2hist[t] = list(h2p)
            emit_psy_copy(nsteps)
    nc.compile()
    return nc


def _prep_inputs(y0, t, W1, b1, W2, b2, W3, b3):
    import ml_dtypes
    bf16 = ml_dtypes.bfloat16
    f64 = np.float64
    dt = float(np.asarray(t)[1] - np.asarray(t)[0])

    w1r = np.ascontiguousarray(W1.reshape(D, 2, 128)).astype(np.float32)
    m31 = (dt * (W3.astype(f64) @ W1.astype(f64))).astype(np.float32)
    w31r = np.ascontiguousarray(
        m31.reshape(2, 128, 2, 128).transpose(1, 0, 2, 3))
    w2r = np.ascontiguousarray(W2.reshape(2, 128, 2, 128).transpose(1, 0, 2, 3))
    w3r = np.ascontiguousarray((dt * W3.astype(f64)).astype(np.float32)
                               .reshape(2, 128, D).transpose(1, 0, 2))
    b1r = np.ascontiguousarray(b1.reshape(2, 128).T).astype(np.float32)
    b2r = np.ascontiguousarray(b2.reshape(2, 128).T).astype(np.float32)
    i64 = np.eye(D, dtype=np.float32)

    has_b3 = bool(np.any(b3 != 0))
    b2_uniform = bool(np.array_equal(b2r[:, 0], b2r[:, 1]))
    crow = (dt * (W1.astype(f64).T @ b3.astype(f64))).astype(np.float32)
    crow = np.ascontiguousarray(crow.reshape(1, 2, 128))
    db3row = (dt * b3.astype(f64)).astype(np.float32).reshape(1, D)

    base = {"w1": w1r, "i64": i64,
            "w31": w31r.astype(bf16), "w2": w2r.astype(bf16),
            "w3": w3r.astype(bf16), "b1r": b1r, "b2r": b2r}
    if has_b3:
        base["crow"] = crow
        base["db3row"] = db3row
    in_maps = []
    for c in range(NCORES):
        y0T = np.ascontiguousarray(y0[c * BL:(c + 1) * BL].T).astype(np.float32)
        m = dict(base)
        m["y0T"] = y0T
        in_maps.append(m)
    return in_maps, has_b3, b2_uniform


def kernel(y0, t, W1, b1, W2, b2, W3, b3, nwarm0: int = 24, nfill: int = 0,
           **run_kwargs):
    nsteps = int(t.shape[0]) - 1
    in_maps, has_b3, b2_uniform = _prep_inputs(y0, t, W1, b1, W2, b2, W3, b3)
    key = (nsteps, nwarm0, nfill, has_b3, b2_uniform)
    if key not in _cache:
        _cache[key] = build(nsteps, nwarm0, nfill, has_b3, b2_uniform)
    nc = _cache[key]
    res = run_bass_kernel_spmd(nc, in_maps, core_ids=list(range(NCORES)),
                               **run_kwargs)
    parts = []
    for c in range(NCORES):
        oc = res.results[c]["out"]            # [nsteps, D, BL]
        parts.append(np.ascontiguousarray(oc.transpose(2, 0, 1)))  # [BL,ns,D]
    full = np.concatenate(parts, axis=0)      # [B, nsteps, D]
    out = np.concatenate([y0[:, None, :].astype(np.float32), full], axis=1)
    return out
